# revision 6
# baseline (speedup 1.0000x reference)
"""Trainium2 Bass kernel for DifferentiableLandmarkDetector (top-k soft-argmax).

Full input: heatmap [2, 16, 96, 128, 128] f32.  For each of the 32 (B, C)
slices: top-64 over the flattened 1,572,864-voxel volume, temperature softmax
over the 64 values, probability-weighted (d, h, w) coordinate sum -> [2,16,3].

Strategy (memory-bound regime):
  - Shard the 32 independent (B,C) slices across 8 cores (4 slices = 25.2MB
    per core, contiguous in HBM).
  - Device kernel: stream the shard through SBUF in 2MB tiles and max-reduce
    every 64 contiguous voxels (DVE tensor_reduce) -> 98304 group maxes
    (393KB) DMA'd out.  This is the single full read of HBM (roofline pass).
  - Host epilogue (O(100KB) of data): at most 64 groups can contain a top-64
    element (each such group's max >= the 64th largest value), so the top-128
    groups by group-max provably contain the entire top-64 set.  Gather those
    128*64 candidates from the input, exact top-64 (jax.lax.top_k tie
    semantics), softmax + coordinate decode in numpy.
"""

import sys

import numpy as np

if "/opt/trn_rl_repo" not in sys.path:
    sys.path.insert(0, "/opt/trn_rl_repo")

TEMPERATURE = 0.1
TOPK = 64
B, C, D, H, W = 2, 16, 96, 128, 128
VOX = D * H * W                          # 1,572,864 voxels per (B,C) slice
N_CORES = 8
SLICES_PER_CORE = (B * C) // N_CORES     # 4
CORE_ELEMS = SLICES_PER_CORE * VOX       # 6,291,456
P = 128                                  # SBUF partitions
TILE_F = 4096                            # free-dim elems per tile (2MB tiles)
TILE_ELEMS = P * TILE_F                  # 524,288
N_TILES = CORE_ELEMS // TILE_ELEMS       # 12
GROUP = 64                               # contiguous voxels per group-max
GROUPS_PER_PART = TILE_F // GROUP        # 64
GM_COLS = N_TILES * GROUPS_PER_PART      # 768
GROUPS_PER_SLICE = VOX // GROUP          # 24,576
TOP_GROUPS = 128                         # >= 64 guarantees exactness

# Set by a caller (e.g. test harness) to profile; LAST_RESULTS then holds the
# BassKernelResults with exec_time_ns.
PROFILE = False
LAST_RESULTS = None

_nc_cache = None


def _build_nc():
    global _nc_cache
    if _nc_cache is not None:
        return _nc_cache
    from concourse import bacc, mybir
    from concourse.tile import TileContext

    nc = bacc.Bacc()
    x = nc.declare_dram_parameter(
        "x", [CORE_ELEMS], mybir.dt.float32, isOutput=False
    )
    gm = nc.declare_dram_parameter(
        "gm", [N_TILES, P, GROUPS_PER_PART], mybir.dt.float32, isOutput=True
    )
    xt = x[:].rearrange("(t p f) -> t p f", p=P, f=TILE_F)

    with TileContext(nc) as tc:
        with (
            tc.tile_pool(name="data", bufs=4) as pool,
            tc.tile_pool(name="gmp", bufs=N_TILES) as gpool,
        ):
            for t in range(N_TILES):
                tl = pool.tile([P, TILE_F], mybir.dt.float32)
                nc.sync.dma_start(out=tl[:], in_=xt[t])
                gm_t = gpool.tile([P, GROUPS_PER_PART], mybir.dt.float32)
                nc.vector.tensor_reduce(
                    out=gm_t[:],
                    in_=tl[:].rearrange("p (g e) -> p g e", e=GROUP),
                    axis=mybir.AxisListType.X,
                    op=mybir.AluOpType.max,
                )
                nc.sync.dma_start(out=gm[t], in_=gm_t[:])
    nc.finalize()
    _nc_cache = nc
    return nc


def kernel(heatmap) -> np.ndarray:
    global LAST_RESULTS
    from concourse.bass_utils import run_bass_kernel_spmd

    x = np.ascontiguousarray(np.asarray(heatmap), dtype=np.float32)
    assert x.shape == (B, C, D, H, W)
    x2 = x.reshape(B * C, VOX)

    nc = _build_nc()
    in_maps = [
        {"x": np.ascontiguousarray(
            x2[i * SLICES_PER_CORE:(i + 1) * SLICES_PER_CORE].reshape(-1))}
        for i in range(N_CORES)
    ]
    res = run_bass_kernel_spmd(
        nc, in_maps, list(range(N_CORES)), trace=PROFILE
    )
    LAST_RESULTS = res

    ecols = np.arange(GROUP)
    out = np.zeros((B * C, 3), dtype=np.float32)
    for core in range(N_CORES):
        # gm[t, p, q] is the max of core-flat elems [(t*8192+p*64+q)*64, +64)
        Gf = res.results[core]["gm"].reshape(-1)  # indexed by core-flat group id
        for s in range(SLICES_PER_CORE):
            bc = core * SLICES_PER_CORE + s
            gs = Gf[s * GROUPS_PER_SLICE:(s + 1) * GROUPS_PER_SLICE]
            top_g = np.argpartition(gs, -TOP_GROUPS)[-TOP_GROUPS:]
            fpos = (top_g[:, None] * GROUP + ecols[None, :]).reshape(-1)
            vals = x2[bc, fpos]
            # descending by value, ties -> lower index (jax.lax.top_k order)
            order = np.lexsort((fpos, -vals))[:TOPK]
            v64 = vals[order].astype(np.float64)
            p64 = fpos[order]
            w = v64 / TEMPERATURE
            w -= w.max()
            ew = np.exp(w)
            probs = ew / (ew.sum() + 1e-20)
            d = p64 // (H * W)
            h = (p64 % (H * W)) // W
            wv = p64 % W
            out[bc, 0] = (probs * d).sum()
            out[bc, 1] = (probs * h).sum()
            out[bc, 2] = (probs * wv).sum()
    return out.reshape(B, C, 3)


# revision 8
# speedup vs baseline: 1.3271x; 1.3271x over previous
"""Trainium2 Bass kernel for DifferentiableLandmarkDetector (top-k soft-argmax).

Full input: heatmap [2, 16, 96, 128, 128] f32.  For each of the 32 (B, C)
slices: top-64 over the flattened 1,572,864-voxel volume, temperature softmax
over the 64 values, probability-weighted (d, h, w) coordinate sum -> [2,16,3].

Strategy (memory-bound regime):
  - Shard the 32 independent (B,C) slices across 8 cores (4 slices = 25.2MB
    per core, contiguous in HBM).
  - Device kernel: stream the shard through SBUF in 2MB tiles and max-reduce
    every 64 contiguous voxels (DVE tensor_reduce) -> 98304 group maxes
    (393KB) DMA'd out.  This is the single full read of HBM (roofline pass).
  - Host epilogue (O(100KB) of data): at most 64 groups can contain a top-64
    element (each such group's max >= the 64th largest value), so the top-128
    groups by group-max provably contain the entire top-64 set.  Gather those
    128*64 candidates from the input, exact top-64 (jax.lax.top_k tie
    semantics), softmax + coordinate decode in numpy.
"""

import sys

import numpy as np

if "/opt/trn_rl_repo" not in sys.path:
    sys.path.insert(0, "/opt/trn_rl_repo")

TEMPERATURE = 0.1
TOPK = 64
B, C, D, H, W = 2, 16, 96, 128, 128
VOX = D * H * W                          # 1,572,864 voxels per (B,C) slice
N_CORES = 8
SLICES_PER_CORE = (B * C) // N_CORES     # 4
CORE_ELEMS = SLICES_PER_CORE * VOX       # 6,291,456
P = 128                                  # SBUF partitions
TILE_F = 4096                            # free-dim elems per tile (2MB tiles)
TILE_ELEMS = P * TILE_F                  # 524,288
N_TILES = CORE_ELEMS // TILE_ELEMS       # 12
GROUP = 64                               # contiguous voxels per group-max
GROUPS_PER_PART = TILE_F // GROUP        # 64
GM_COLS = N_TILES * GROUPS_PER_PART      # 768
GROUPS_PER_SLICE = VOX // GROUP          # 24,576
TOP_GROUPS = 128                         # >= 64 guarantees exactness

# Set by a caller (e.g. test harness) to profile; LAST_RESULTS then holds the
# BassKernelResults with exec_time_ns.
PROFILE = False
LAST_RESULTS = None

_nc_cache = None


def _build_nc():
    global _nc_cache
    if _nc_cache is not None:
        return _nc_cache
    from concourse import bacc, mybir
    from concourse.tile import TileContext

    nc = bacc.Bacc()
    x = nc.declare_dram_parameter(
        "x", [CORE_ELEMS], mybir.dt.float32, isOutput=False
    )
    gm = nc.declare_dram_parameter(
        "gm", [N_TILES, P, GROUPS_PER_PART], mybir.dt.float32, isOutput=True
    )
    xt = x[:].rearrange("(t p f) -> t p f", p=P, f=TILE_F)

    with TileContext(nc) as tc:
        with (
            tc.tile_pool(name="data", bufs=6) as pool,
            tc.tile_pool(name="gmp", bufs=N_TILES) as gpool,
        ):
            for t in range(N_TILES):
                tl = pool.tile([P, TILE_F], mybir.dt.float32)
                nc.sync.dma_start(out=tl[:], in_=xt[t])
                gm_t = gpool.tile([P, GROUPS_PER_PART], mybir.dt.float32)
                nc.vector.tensor_reduce(
                    out=gm_t[:],
                    in_=tl[:].rearrange("p (g e) -> p g e", e=GROUP),
                    axis=mybir.AxisListType.X,
                    op=mybir.AluOpType.max,
                )
                # scalar (ACT) is also HWDGE on TRN2; keeping the tiny gm
                # writes off the SP sequencer keeps its FIFO purely input
                # DMAs, so loads stream back-to-back instead of queuing
                # behind an out-trigger that waits on the reduce.
                nc.scalar.dma_start(out=gm[t], in_=gm_t[:])
    nc.finalize()
    _nc_cache = nc
    return nc


def kernel(heatmap) -> np.ndarray:
    global LAST_RESULTS
    from concourse.bass_utils import run_bass_kernel_spmd

    x = np.ascontiguousarray(np.asarray(heatmap), dtype=np.float32)
    assert x.shape == (B, C, D, H, W)
    x2 = x.reshape(B * C, VOX)

    nc = _build_nc()
    in_maps = [
        {"x": np.ascontiguousarray(
            x2[i * SLICES_PER_CORE:(i + 1) * SLICES_PER_CORE].reshape(-1))}
        for i in range(N_CORES)
    ]
    res = run_bass_kernel_spmd(
        nc, in_maps, list(range(N_CORES)), trace=PROFILE
    )
    LAST_RESULTS = res

    ecols = np.arange(GROUP)
    out = np.zeros((B * C, 3), dtype=np.float32)
    for core in range(N_CORES):
        # gm[t, p, q] is the max of core-flat elems [(t*8192+p*64+q)*64, +64)
        Gf = res.results[core]["gm"].reshape(-1)  # indexed by core-flat group id
        for s in range(SLICES_PER_CORE):
            bc = core * SLICES_PER_CORE + s
            gs = Gf[s * GROUPS_PER_SLICE:(s + 1) * GROUPS_PER_SLICE]
            top_g = np.argpartition(gs, -TOP_GROUPS)[-TOP_GROUPS:]
            fpos = (top_g[:, None] * GROUP + ecols[None, :]).reshape(-1)
            vals = x2[bc, fpos]
            # descending by value, ties -> lower index (jax.lax.top_k order)
            order = np.lexsort((fpos, -vals))[:TOPK]
            v64 = vals[order].astype(np.float64)
            p64 = fpos[order]
            w = v64 / TEMPERATURE
            w -= w.max()
            ew = np.exp(w)
            probs = ew / (ew.sum() + 1e-20)
            d = p64 // (H * W)
            h = (p64 % (H * W)) // W
            wv = p64 % W
            out[bc, 0] = (probs * d).sum()
            out[bc, 1] = (probs * h).sum()
            out[bc, 2] = (probs * wv).sum()
    return out.reshape(B, C, 3)
